# revision 52
# baseline (speedup 1.0000x reference)
"""CoAttLayer Trainium2 kernel (v2).

Data-parallel over batch: 64 batches -> 8 NeuronCores x 8 batches.
Per batch (T = N = 1024, d = 64, k = 128):
    L  = tanh(R @ Wl @ P^T)                      (T, N)
    Hp = tanh(Wp @ P^T + (Wr @ R^T) @ L)         (k, N)
    Hr = tanh(Wr @ R^T + (Wp @ P^T) @ L^T)       (k, T)
    Ap = softmax(whp @ Hp), Ar = softmax(whr @ Hr)
    out = [P^T @ Ap ; R^T @ Ar]                  (2d,)

v2 layout strategy vs v1:
  * Inputs loaded via SWDGE (gpsimd) DMA with an f32->f16 cast on the
    fly, using the contiguous "(p i) d" split (row 8p+i on partition p)
    -- 2KB descriptors instead of 256B.  The global t/n permutation this
    introduces cancels everywhere (softmax pools are order-free).
  * The whole datapath below the loads is fp16.
  * L^T is produced by the DMA xbar transpose (dma_start_transpose, one
    contiguous [128, NN*128] plane per L t-tile) instead of 64 PE
    transposes + DVE evacuation per batch.  CRITICAL: regular DMA must
    stay OFF the HWDGE queue (hence SWDGE loads/stores) -- mixing
    in-flight HWDGE copies with xbar transposes corrupts the transpose
    output on hardware (observed: even partitions rounded to ~4-bit
    mantissa).  Non-contiguous xbar destinations are also silently wrong.
  * Gr^T = R @ Wr^T and Gp^T = P @ Wp^T computed directly on the PE
    (Rt/Pt tiles as the stationary operand) -- same PE cost as the
    untransposed products, no transpose step at all.
  * L = R @ (Wl P^T): the small Wl contraction is folded into the P side
    (C = Wl @ P^T once per batch), so L's stationary tiles come straight
    from Rt.
  * Hp accumulation is interleaved with L-tile production pair-by-pair
    so ACT (tanh) and PE overlap tightly; Hr/logits/pool run as a
    trailing phase one pipeline slot behind.
"""

import numpy as np
from contextlib import ExitStack

B, T, N, D, K = 64, 1024, 1024, 64, 128
NCORES = 8
BL = B // NCORES  # batches per core

_CACHE = {}


def _build():
    import concourse.tile as tile
    from concourse import bacc, mybir
    from concourse.masks import make_identity

    f32 = mybir.dt.float32
    f32r = mybir.dt.float32r
    f16 = mybir.dt.float16
    f8 = mybir.dt.float8e4
    DR = mybir.MatmulPerfMode.DoubleRow
    Tanh = mybir.ActivationFunctionType.Tanh
    Exp = mybir.ActivationFunctionType.Exp

    nc = bacc.Bacc(trn_type="TRN2")

    rv = nc.dram_tensor("review_seq", (BL, T, D), f32r, kind="ExternalInput")
    po = nc.dram_tensor("post_seq", (BL, N, D), f32r, kind="ExternalInput")
    wl = nc.dram_tensor("Wl", (D, D), f32r, kind="ExternalInput")
    wr = nc.dram_tensor("Wr", (K, D), f32r, kind="ExternalInput")
    wp = nc.dram_tensor("Wp", (K, D), f32r, kind="ExternalInput")
    whr = nc.dram_tensor("whr", (1, K), f32, kind="ExternalInput")
    whp = nc.dram_tensor("whp", (1, K), f32, kind="ExternalInput")
    out = nc.dram_tensor("out", (BL, 2 * D), f32, kind="ExternalOutput")
    # Version-nonce output: its shape encodes a hash of this function's
    # source, so the lowered program (and any executable cache keyed on it)
    # changes whenever the kernel changes.
    import inspect
    import hashlib
    _sig = hashlib.sha256(inspect.getsource(_build).encode()).digest()
    _nonce = (int.from_bytes(_sig[:4], "little") % 509) + 1
    nc.dram_tensor("ver", (1, _nonce), f32, kind="ExternalOutput")
    import os
    DBG = bool(int(os.environ.get("KBDBG", "0")))
    if DBG:
        f16_ = mybir.dt.float16
        dbg_lf = nc.dram_tensor("dbg_lf", (BL, 128, 8, 1024), f16_, kind="ExternalOutput")
        dbg_lt = nc.dram_tensor("dbg_lt", (BL, 128, 8, 8, 128), f16_, kind="ExternalOutput")
        dbg_gpt = nc.dram_tensor("dbg_gpt", (BL, 128, 8, 128), f16_, kind="ExternalOutput")
        dbg_hr = nc.dram_tensor("dbg_hr", (BL, 128, 1024), f16_, kind="ExternalOutput")
        dbg_hp = nc.dram_tensor("dbg_hp", (BL, 128, 1024), f16_, kind="ExternalOutput")
        dbg_ee = nc.dram_tensor("dbg_ee", (BL, 128, 16), f16_, kind="ExternalOutput")

    NT = T // 128  # 8 t-tiles
    NN = N // 128  # 8 n-tiles

    with tile.TileContext(nc) as tc, ExitStack() as ctx:
        singles = ctx.enter_context(tc.tile_pool(name="singles", bufs=1))
        sb = ctx.enter_context(tc.tile_pool(name="sb", bufs=2))
        pa = ctx.enter_context(tc.tile_pool(name="pa", bufs=2, space="PSUM"))
        ph = ctx.enter_context(tc.tile_pool(name="ph", bufs=1, space="PSUM"))
        pr = ctx.enter_context(tc.tile_pool(name="pr", bufs=1, space="PSUM"))

        # ---- per-core constants -------------------------------------------
        st = {}
        ident32 = singles.tile([128, 128], f32)
        make_identity(nc, ident32)
        ident16 = singles.tile([128, 128], f16)
        nc.vector.tensor_copy(ident16, ident32)
        one11 = singles.tile([1, 1], f32)
        nc.vector.memset(one11, 1.0)

        wl_32 = singles.tile([64, 64], f32r)
        nc.sync.dma_start(out=wl_32, in_=wl[:, :])
        wr_sb = singles.tile([128, 64], f32r)
        nc.sync.dma_start(out=wr_sb, in_=wr[:, :])
        wp_sb = singles.tile([128, 64], f32r)
        nc.sync.dma_start(out=wp_sb, in_=wp[:, :])
        whp_sb = singles.tile([1, 128], f32)
        nc.sync.dma_start(out=whp_sb, in_=whp[:, :])
        whr_sb = singles.tile([1, 128], f32)
        nc.sync.dma_start(out=whr_sb, in_=whr[:, :])

        # Wr^T, Wp^T on partitions 0-63 (fp16); whp^T/whr^T as fp16 columns.
        ident = singles.tile([128, 128], f32r)
        nc.vector.tensor_copy(ident, ident32)
        ps_w = pa.tile([128, 1024], f32r, tag="pa")
        nc.tensor.transpose(ps_w[0:64, 0:128], wr_sb, ident)
        nc.tensor.transpose(ps_w[0:64, 128:256], wp_sb, ident)
        nc.tensor.transpose(ps_w[0:64, 256:320], wl_32, ident[0:64, 0:64])
        wrT = singles.tile([64, 128], f16)
        nc.vector.tensor_copy(wrT, ps_w[0:64, 0:128])
        wpT = singles.tile([64, 128], f16)
        nc.vector.tensor_copy(wpT, ps_w[0:64, 128:256])
        wlT = singles.tile([64, 64], f16)
        nc.vector.tensor_copy(wlT, ps_w[0:64, 256:320])
        ps_wh = pa.tile([128, 2], f32, tag="pa")
        nc.tensor.transpose(ps_wh[0:128, 0:1], whp_sb, one11)
        nc.tensor.transpose(ps_wh[0:128, 1:2], whr_sb, one11)
        whT = singles.tile([128, 2], f16)
        nc.vector.tensor_copy(whT, ps_wh)

        # ---- per-batch pipeline, software-pipelined emission ---------------

        def phaseA0(b):
            # SWDGE (gpsimd) loads with f32->f16 cast on the fly.  Keeping all
            # regular DMA off the HWDGE queue isolates the xbar transposes,
            # which corrupt when mixed with in-flight HWDGE copies.
            s = st[b] = {}
            s["RP16"] = RP16 = sb.tile(name="rp16", shape=[128, NT, 64], dtype=f16, tag="rp16", bufs=3)
            nc.gpsimd.dma_start(out=RP16, in_=rv[b, :, :].rearrange("(p i) d -> p i d", p=128))
            s["PP16"] = PP16 = sb.tile(name="pp16", shape=[128, NN, 64], dtype=f16, tag="pp16", bufs=3)
            nc.gpsimd.dma_start(out=PP16, in_=po[b, :, :].rearrange("(p i) d -> p i d", p=128))

        def phaseA1(b):
            if b not in st:
                phaseA0(b)
            s = st[b]
            RP16, PP16 = s["RP16"], s["PP16"]

            ps_rt = pa.tile([128, 1024], f16, tag="pa")
            for i in range(NT):
                nc.tensor.transpose(ps_rt[0:64, 128 * i:128 * (i + 1)], RP16[:, i, :], ident16)
            s["Rt"] = Rt = sb.tile(name="rt", shape=[64, 1024], dtype=f16, tag="rt", bufs=3)
            nc.vector.tensor_copy(Rt, ps_rt[0:64, :])

            ps_pt = pa.tile([128, 1024], f16, tag="pa")
            for i in range(NN):
                nc.tensor.transpose(ps_pt[0:64, 128 * i:128 * (i + 1)], PP16[:, i, :], ident16)
            s["Pt"] = Pt = sb.tile(name="pt", shape=[64, 1024], dtype=f16, tag="pt", bufs=3)
            nc.vector.tensor_copy(Pt, ps_pt[0:64, :])

            # pooling rhs with ones column (gpsimd: off the DVE)
            s["Pe"] = Pe = sb.tile(name="pe", shape=[128, NN, 65], dtype=f16, tag="pe", bufs=3)
            nc.gpsimd.tensor_copy(out=Pe[:, :, 0:64], in_=PP16)
            nc.gpsimd.memset(Pe[:, :, 64:65], 1.0)
            s["Re"] = Re = sb.tile(name="re", shape=[128, NT, 65], dtype=f16, tag="re", bufs=3)
            nc.gpsimd.tensor_copy(out=Re[:, :, 0:64], in_=RP16)
            nc.gpsimd.memset(Re[:, :, 64:65], 1.0)

        def phaseA2(b):
            s = st[b]
            Rt, Pt = s["Rt"], s["Pt"]
            # C = Wl @ P^T -- L is then R @ C with Rt tiles as the stationary
            # operand (the former A^T stage is folded into the P side).
            ps_c = pa.tile([128, 1024], f32, tag="pa")
            nc.tensor.matmul(ps_c[0:64, 0:512], wlT, Pt[:, 0:512], start=True, stop=True)
            nc.tensor.matmul(ps_c[0:64, 512:1024], wlT, Pt[:, 512:1024], start=True, stop=True)
            s["Cw"] = Cw = sb.tile(name="cw", shape=[64, 1024], dtype=f16, tag="cw", bufs=3)
            nc.vector.tensor_copy(Cw, ps_c[0:64, :])

            # Gr^T = R @ Wr^T computed directly (Rt tiles as the stationary
            # operand) -- no untransposed Gr needed, no xbar DMA.
            ps_grt = pa.tile([128, 1024], f32, tag="pa")
            for j in range(NT):
                nc.tensor.matmul(ps_grt[:, 128 * j:128 * (j + 1)],
                                 Rt[:, 128 * j:128 * (j + 1)], wrT,
                                 start=True, stop=True)
            s["GrT"] = GrT = sb.tile(name="grt", shape=[128, NT, 128], dtype=f16, tag="grt", bufs=3)
            nc.vector.tensor_copy(GrT, ps_grt)

            # Gp^T = P @ Wp^T computed directly on the PE (the prologue-window
            # xbar transposes were observed to corrupt on HW; the PE route
            # costs the same cycles as producing Gp untransposed).
            ps_gpt = pa.tile([128, 1024], f32, tag="pa")
            for j in range(NN):
                nc.tensor.matmul(ps_gpt[:, 128 * j:128 * (j + 1)],
                                 Pt[:, 128 * j:128 * (j + 1)], wpT,
                                 start=True, stop=True)
            s["GpT"] = GpT = sb.tile(name="gpt", shape=[128, NN, 128], dtype=f16, tag="gpt", bufs=3)
            nc.vector.tensor_copy(GpT, ps_gpt)

        def phaseB1(b, trail=None):
            """L production + Hp accumulation for batch b, with the chunks of
            the previous batch's Hr/output phases (``trail``) interleaved
            between pairs so PE never waits on ACT."""
            s = st[b]
            Rt, Pt, Cw, GrT = s["Rt"], s["Pt"], s["Cw"], s["GrT"]
            ps_hp = ph.tile([128, 1024], f32, tag="hp", name="ps_hp")
            nc.tensor.matmul(ps_hp[:, 0:512], wpT, Pt[:, 0:512], start=True, stop=False)
            nc.tensor.matmul(ps_hp[:, 512:1024], wpT, Pt[:, 512:1024], start=True, stop=False)
            s["Lf"] = Lf = sb.tile(name="lf", shape=[128, NT, 1024], dtype=f16, tag="lf")
            # LT[p, a, j, t'] = L[128a+t', 128j+p]; each xbar transpose of an
            # L t-tile writes one contiguous [128, NN*128] plane (a
            # non-contiguous xbar destination is silently wrong on HW).
            s["LT"] = LT = sb.tile(name="lt", shape=[128, NT, NN, 128], dtype=f16, tag="lt")
            trail = trail or []

            def hp_accum(p, last=False):
                for a in (2 * p, 2 * p + 1):
                    fin = last and a == 2 * p + 1
                    nc.tensor.matmul(ps_hp[:, 0:512], GrT[:, a, :], Lf[:, a, 0:512],
                                     start=False, stop=fin)
                    nc.tensor.matmul(ps_hp[:, 512:1024], GrT[:, a, :], Lf[:, a, 512:1024],
                                     start=False, stop=fin)

            for p in range(NT // 2):
                if p > 0 and trail:
                    trail.pop(0)()
                for q in range(2):
                    a = 2 * p + q
                    ps_l = pa.tile([128, 1024], f32, tag="pa")
                    lhs = Rt[:, 128 * a:128 * (a + 1)]
                    nc.tensor.matmul(ps_l[:, 0:512], lhs, Cw[:, 0:512], start=True, stop=True)
                    nc.tensor.matmul(ps_l[:, 512:1024], lhs, Cw[:, 512:1024], start=True, stop=True)
                    nc.scalar.activation(Lf[:, a, :], ps_l, Tanh)
                    nc.sync.dma_start_transpose(out=LT[:, a], in_=Lf[:, a, :])
                if p > 0:
                    hp_accum(p - 1)
            hp_accum(NT // 2 - 1, last=True)
            while trail:
                trail.pop(0)()
            s["Hp16"] = Hp16 = sb.tile(name="hp16", shape=[128, 1024], dtype=f16, tag="hp16")
            nc.scalar.activation(Hp16, ps_hp, Tanh)

        def tail_chunks(b):
            """Hr accumulation + logits/softmax/pool for batch b as four
            closures, to be interleaved into the next batch's B1."""
            s = st[b]

            def c1():
                Rt = s["Rt"]
                ps_hr = pr.tile([128, 1024], f32, tag="hr", name="ps_hr")
                s["ps_hr"] = ps_hr
                nc.tensor.matmul(ps_hr[:, 0:512], wrT, Rt[:, 0:512], start=True, stop=False)
                nc.tensor.matmul(ps_hr[:, 512:1024], wrT, Rt[:, 512:1024], start=True, stop=False)

            def c2():
                GpT, LT = s["GpT"], s["LT"]
                ps_hr = s["ps_hr"]
                for j in range(NN):
                    nc.tensor.matmul(ps_hr[:, 0:512], GpT[:, j, :], LT[:, 0:4, j, :],
                                     start=False, stop=(j == NN - 1))
                    nc.tensor.matmul(ps_hr[:, 512:1024], GpT[:, j, :], LT[:, 4:8, j, :],
                                     start=False, stop=(j == NN - 1))
                s["Hr16"] = Hr16 = sb.tile(name="hr16", shape=[128, 1024], dtype=f16, tag="hr16")
                nc.scalar.activation(Hr16, ps_hr, Tanh)

            def c3():
                Hp16, Hr16 = s["Hp16"], s["Hr16"]
                ps_lg = pr.tile([128, 16], f32, tag="hr", name="ps_lg")
                s["ps_lg"] = ps_lg
                for i in range(NN):
                    nc.tensor.matmul(ps_lg[:, i:i + 1], Hp16[:, 128 * i:128 * (i + 1)],
                                     whT[:, 0:1], start=True, stop=True)
                for i in range(NT):
                    nc.tensor.matmul(ps_lg[:, 8 + i:9 + i], Hr16[:, 128 * i:128 * (i + 1)],
                                     whT[:, 1:2], start=True, stop=True)
                s["ee"] = ee = sb.tile([128, 16], f16, tag="ee", name="ee")
                nc.scalar.activation(ee, ps_lg, Exp)

            def c4():
                Pe, Re, ee = s["Pe"], s["Re"], s["ee"]
                ps_co = pr.tile([128, 1024], f32, tag="hr", name="ps_co")
                for j in range(NN):
                    nc.tensor.matmul(ps_co[0:1, 0:65], ee[:, j:j + 1], Pe[:, j, :],
                                     start=(j == 0), stop=(j == NN - 1))
                for j in range(NT):
                    nc.tensor.matmul(ps_co[0:1, 512:577], ee[:, 8 + j:9 + j], Re[:, j, :],
                                     start=(j == 0), stop=(j == NT - 1))
                if DBG:
                    nc.sync.dma_start(out=dbg_ee[b], in_=ee)
                    nc.sync.dma_start(out=dbg_lf[b], in_=s["Lf"])
                    nc.sync.dma_start(out=dbg_lt[b], in_=s["LT"])
                    nc.sync.dma_start(out=dbg_gpt[b], in_=s["GpT"])
                    nc.sync.dma_start(out=dbg_hr[b], in_=s["Hr16"])
                    nc.sync.dma_start(out=dbg_hp[b], in_=s["Hp16"])
                rinv = sb.tile([1, 2], f32, tag="rinv", name="rinv")
                nc.vector.reciprocal(rinv[0:1, 0:1], ps_co[0:1, 64:65])
                nc.vector.reciprocal(rinv[0:1, 1:2], ps_co[0:1, 576:577])
                ob = sb.tile([1, 128], f32, tag="ob", name="ob")
                nc.vector.tensor_scalar_mul(ob[0:1, 0:64], ps_co[0:1, 0:64], rinv[0:1, 0:1])
                nc.vector.tensor_scalar_mul(ob[0:1, 64:128], ps_co[0:1, 512:576], rinv[0:1, 1:2])
                nc.gpsimd.dma_start(out=out[b:b + 1, :], in_=ob)
                st.pop(b)

            return [c1, c2, c3, c4]

        # Prologue + steady-state interleave.  A-phases lead B by two
        # batches; batch k's Hr/output chunks ride inside batch k+1's B1.
        phaseA1(0)
        if BL > 1:
            phaseA1(1)
            phaseA2(0)
            phaseA2(1)
            phaseB1(0)
            for k in range(BL):
                if k + 2 < BL:
                    phaseA1(k + 2)
                if k + 1 < BL:
                    phaseB1(k + 1)
                if k + 2 < BL:
                    phaseA2(k + 2)
                for c in tail_chunks(k):
                    c()
        else:
            phaseA2(0)
            phaseB1(0)
            for c in tail_chunks(0):
                c()

    nc.compile()
    return nc


def get_nc():
    if "nc" not in _CACHE:
        _CACHE["nc"] = _build()
    return _CACHE["nc"]


def make_in_maps(inputs):
    R = np.ascontiguousarray(inputs["review_seq"], dtype=np.float32)
    P = np.ascontiguousarray(inputs["post_seq"], dtype=np.float32)
    w = {
        "Wl": np.ascontiguousarray(inputs["Wl"], dtype=np.float32),
        "Wr": np.ascontiguousarray(inputs["Wr"], dtype=np.float32),
        "Wp": np.ascontiguousarray(inputs["Wp"], dtype=np.float32),
        "whr": np.ascontiguousarray(inputs["whr"], dtype=np.float32),
        "whp": np.ascontiguousarray(inputs["whp"], dtype=np.float32),
    }
    in_maps = []
    for c in range(NCORES):
        m = {
            "review_seq": np.ascontiguousarray(R[c * BL:(c + 1) * BL]),
            "post_seq": np.ascontiguousarray(P[c * BL:(c + 1) * BL]),
        }
        m.update(w)
        in_maps.append(m)
    return in_maps


def run(inputs, trace=False):
    from concourse.bass_utils import run_bass_kernel_spmd

    nc = get_nc()
    res = run_bass_kernel_spmd(nc, make_in_maps(inputs),
                               core_ids=list(range(NCORES)), trace=trace)
    outp = np.concatenate([r["out"] for r in res.results], axis=0)
    return outp.astype(np.float32), res


def kernel(**inputs) -> np.ndarray:
    outp, _ = run(inputs, trace=False)
    return outp
